# revision 42
# baseline (speedup 1.0000x reference)
"""BENDR contrastive-loss kernel for Trainium2 (8 NeuronCores).

Reference computation (see problem): for each (b, t):
  logits[b*T+t, 0]   = cos(z[b,:,t], c[b,:,t+1]) / TEMP
  logits[b*T+t, 1+k] = cos(z[b,:,t], z[b,:,n(b,t,k)]) / TEMP
with n(b,t,k) = negative_inds[b, t*K+k] (row-local), TEMP=0.5.

Strategy: data-parallel over batch (2 rows per core).  Every negative logit
is an entry of the symmetric Gram matrix G = z^T z (z columns = feature
vectors) scaled by 2/(|z_t||z_j|), and the norms are G's own diagonal.  The
device is a PURE Gram kernel: it computes the UPPER-TRIANGLE 128-row blocks
of G in fp8-e4m3 with DoubleRow matmuls (256-deep contraction in one pass,
2 MACs/cell/cycle) and ships them as fp16.  Everything O(B*T*F) or smaller
-- the positive column 2*u[t]/(|z_t||c_t|), the c-norms, the normalize and
the index-pick gather -- runs on the host (0.1% of the FLOPs; numpy).

Normalizing the negatives by the fp8 Gram's own diagonal is what makes fp8
viable: logits become exact cosines of the QUANTIZED vectors, so the
correlated quantization error cancels (measured rel-err 1.16e-2 against the
2e-2 gate; with exact norms instead it fails at 2.2e-2).  Self-hits
(n == t) become exactly 2.0 automatically.

Per-core timeline: z8 loads in 512-col chunks (GPSIMD-issued triggers, the
sync queue carries the 16 output DMAs); tau pairs stream through a 6-deep
PSUM ring, PSUM->SBUF fp16 evacuation alternates DVE/ACT; two consecutive
tau blocks share one 3D pair-DMA (the second block left-padded 128 junk
cols, never read by the host) into a partition-major DRAM layout.

On-device gathers were measured and rejected: GPSIMD indirect_copy ~29us
per 1024 indices, indirect DMA ~62ns/row -- computing the full Gram block
on the PE and shipping fp16 is far cheaper.
"""

import sys

for _p in ("/opt/trn_rl_repo",):
    if _p not in sys.path:
        sys.path.append(_p)

import numpy as np
import ml_dtypes

import concourse.bass as bass
import concourse.mybir as mybir
from concourse import tile as _tile
from concourse.tile import TileContext
from concourse.bass_utils import run_bass_kernel_spmd

dt = mybir.dt


B, F, T, K = 16, 256, 2048, 20
NCORES = 8
ROWS = B // NCORES          # batch rows per core
NBLK = T // 128             # t-blocks per batch row
FCH = F // 128              # f chunks (partition dim)
EPS = 1e-8

# ---------------------------------------------------------------------------
# Walrus in this container rejects instructions that carry more than one
# semaphore wait ("Too many sync wait commands").  Two shims fix that: the
# tile tail drain gets its waits on single-wait NOPs, and a post-pass splits
# any remaining multi-wait instruction.
# ---------------------------------------------------------------------------


def _patched_drain_and_barrier(self, tick_clock, wait_clock):
    nop0 = self.nc.sync.nop(nofuse=True, hint="tail_wait")
    wait_clock.add_sem_waits(
        nop0.ins, _tile.ScopedClock({None: tick_clock.global_clock})
    )
    si = nop0.ins.sync_info
    if si is not None and len(si.on_wait) > 1:
        waits = list(si.on_wait)
        nop0.ins.sync_info = mybir.SyncInfo(
            on_wait=waits[:1], on_update=list(si.on_update)
        )
        for w in waits[1:]:
            nopi = self.nc.sync.nop(nofuse=True, hint="tail_wait")
            nopi.ins.sync_info = mybir.SyncInfo(on_wait=[w], on_update=[])
    self.nc.sync.drain()
    self.nc.all_engine_barrier()
    assert self.sems is not None
    popped = self.nc._tile_sem_poison_stack.pop()
    assert popped is self._sem_poison
    self.nc.clear_and_free_semaphores(list(self.sems.allocated().values()))
    self.nc.all_engine_barrier()


_tile.TileContext._drain_and_barrier = _patched_drain_and_barrier

_wnop_counter = [0]


def split_excess_waits(nc, cap=1):
    for f in nc.m.functions:
        for bb in f.blocks:
            insts = bb.instructions
            out = []
            changed = False
            for inst in list(insts):
                si = getattr(inst, "sync_info", None)
                waits = list(si.on_wait) if si is not None else []
                if len(waits) > cap:
                    keep = waits[-cap:]
                    for w in waits[: len(waits) - cap]:
                        _wnop_counter[0] += 1
                        nop = mybir.InstNoOp(
                            name=f"wnop-{_wnop_counter[0]}", ins=[], outs=[]
                        )
                        nop.engine = inst.engine
                        nop.sync_info = mybir.SyncInfo(on_wait=[w], on_update=[])
                        out.append(nop)
                    inst.sync_info = mybir.SyncInfo(
                        on_wait=keep, on_update=list(si.on_update)
                    )
                    changed = True
                out.append(inst)
            if changed:
                insts[:] = out


def dedup_ldweights(nc):
    """The tile lowering emits an explicit InstLdweights before every
    InstMatmult.  Consecutive matmuls that share the stationary operand
    (same AP + tile position) don't need the reload -- the PE keeps its
    weights.  Convert redundant loads into NoOps (keeping their sync info)."""
    n = 0
    for f in nc.m.functions:
        for bb in f.blocks:
            insts = bb.instructions
            last_key = None
            out = []
            changed = False
            for inst in list(insts):
                tn = type(inst).__name__
                if tn == "InstLdweights":
                    key = (
                        str(inst.ins[0]),
                        tuple(inst.tile_position or ()),
                        tuple(inst.tile_size or ()),
                        bool(inst.is_transpose),
                    )
                    if key == last_key:
                        nop = mybir.InstNoOp(name=f"ldwnop-{n}", ins=[], outs=[])
                        n += 1
                        nop.engine = inst.engine
                        si = inst.sync_info
                        if si is not None:
                            nop.sync_info = mybir.SyncInfo(
                                on_wait=list(si.on_wait), on_update=list(si.on_update)
                            )
                        out.append(nop)
                        changed = True
                        continue
                    last_key = key
                elif tn == "InstMatmult":
                    if inst.is_transpose:
                        last_key = None
                out.append(inst)
            if changed:
                insts[:] = out
    return n


# ---------------------------------------------------------------------------
# Device program: pure fp8 upper-triangle Gram
# ---------------------------------------------------------------------------


def build_program():
    nc = bass.Bass("TRN2", num_devices=NCORES)
    # z8[r, p, ko, t] = z[r, ko*128 + p, t] as fp8 e4m3 -- the layout the
    # DoubleRow matmul wants ([K=128 partitions, Ko=2, free]).
    z8_in = nc.dram_tensor(
        "z8", [ROWS, 128, FCH, T], dt.float8e4, kind="ExternalInput"
    )
    # upper-triangle Gram blocks, PARTITION-MAJOR: g[p, r*NBLK+tau, j] =
    # G[128*tau + p, j] (valid for j >= 128*tau).
    g_out = nc.dram_tensor(
        "g", [128, ROWS * NBLK, T], dt.float16, kind="ExternalOutput"
    )

    # ---- early input loads, BEFORE the tile-context preamble ----
    # The framework preamble (engine barriers, register/const loads) costs
    # ~7.5us before the first in-context instruction can issue; raw
    # dma_starts emitted here land at the head of the sync queue instead,
    # so z8 is in SBUF by the time the PE clears its preamble.  Completion
    # is signalled on hand-rolled semaphores ('sem-add-imm' +16 at DMA
    # completion, the DGE convention); inject_z8_gates() below adds the
    # matching waits to the first PE consumers after scheduling.
    tiles = {}
    sems = {}
    for r in range(ROWS):
        tiles[r] = nc.alloc_sbuf_tensor(f"z8r{r}", [128, FCH, T], dt.float8e4)
        sems[r] = nc.alloc_semaphore(f"z8sem{r}")
        ins = nc.sync.dma_start(out=tiles[r][:, :, :], in_=z8_in[r])
        ins.ins.sync_info = mybir.SyncInfo(
            on_wait=[],
            on_update=[
                mybir.SyncUpdate(
                    sync_type="semaphore",
                    id=sems[r].num,
                    update_mode="sem-add-imm",
                    update_value=16,
                    ant_name=f"z8r{r}_loaded",
                )
            ],
        )

    with TileContext(nc) as tc:
        with (
            tc.tile_pool(name="outp", bufs=1) as outp,
            tc.tile_pool(name="gram_ps", bufs=6, space="PSUM") as gram_ps,
        ):

            # manual ring of 6 pair-otiles ([t-block 2k | t-block 2k+1]; the
            # second block is left-padded 128 junk cols so one 3D DMA covers
            # both blocks with a single column base).  6 deep because the
            # early pair DMAs are ~2MB / ~5us: with fewer slots the
            # evacuation (and then the PE, via the PSUM ring) stalls on the
            # write-after-read of a slot still being shipped out.
            NOR = 6
            oring = [
                outp.tile([128, 2, T], dt.float16, name=f"ot{i}", tag=f"ot{i}")
                for i in range(NOR)
            ]
            evac_flip = [0]

            def emit_gram_tau(r, tau, ot, ko):
                """Matmuls + PSUM evacuation for one tau block into half `ko`
                of the pair otile `ot` (left-padded 128 cols when ko=1)."""
                z8 = tiles[r]
                t0 = 128 * tau
                w = T - t0
                nch = (w + 511) // 512
                lhsT = z8[:, :, t0 : t0 + 128]
                pts = []
                for c in range(nch):
                    pts.append(
                        gram_ps.tile([128, 512], dt.float32, name="gps", tag="gps")
                    )
                for c in range(nch):
                    cw = min(512, w - 512 * c)
                    c0 = t0 + 512 * c
                    nc.tensor.matmul(
                        pts[c][:, :cw], lhsT, z8[:, :, c0 : c0 + cw],
                        start=True, stop=True,
                        perf_mode=mybir.MatmulPerfMode.DoubleRow,
                    )
                pad = 128 * ko
                for c in range(nch):
                    cw = min(512, w - 512 * c)
                    dst = ot[:, ko, pad + 512 * c : pad + 512 * c + cw]
                    # DVE's PSUM->fp16 cast measures ~504ns vs ACT's ~590ns
                    # per 512 cols: give DVE 6 of every 11 chunks.
                    if (evac_flip[0] * 6) % 11 < 6:
                        nc.vector.tensor_copy(dst, pts[c][:, :cw])
                    else:
                        nc.scalar.copy(dst, pts[c][:, :cw])
                    evac_flip[0] += 1

            # Interleave the two rows' pairs, big-W first: row-serial order
            # emitted row 1's ~4.4MB of output in the last third of the
            # stream, leaving a ~10us DMA drain tail.  Row 1 trails by 2
            # slots so its (early-loaded) z8 is ready when needed.
            order = []
            for i in range(NBLK // 2 + 2):
                if i < NBLK // 2:
                    order.append((0, i))
                if 0 <= i - 2 < NBLK // 2:
                    order.append((1, i - 2))
            for gp, (r, pair) in enumerate(order):
                ot = oring[gp % NOR]
                emit_gram_tau(r, 2 * pair, ot, 0)
                # ship each tau separately (no 128 junk cols), the first as
                # soon as its evacuation is queued; alternate trigger queues
                # (sync / the otherwise-idle gpsimd) to halve the ~790ns
                # serial trigger cost per queue.
                wa = T - 256 * pair
                blk = r * NBLK + 2 * pair
                nc.sync.dma_start(
                    out=g_out[:, blk : blk + 1, 256 * pair :],
                    in_=ot[:, 0:1, :wa],
                )
                emit_gram_tau(r, 2 * pair + 1, ot, 1)
                nc.gpsimd.dma_start(
                    out=g_out[:, blk + 1 : blk + 2, 256 * pair + 128 :],
                    in_=ot[:, 1:2, 128:wa],
                )

    inject_z8_gates(nc, sems)
    dedup_ldweights(nc)
    split_excess_waits(nc)
    return nc


def inject_z8_gates(nc, sems):
    """Add a semaphore wait for each early z8 load to its FIRST PE consumer
    (in final scheduled order).  The PE queue is in-order, so one gate per
    row covers every later matmul; split_excess_waits() handles the 1-wait
    ISA cap if the instruction already carries a wait."""
    gated = set()
    for f in nc.m.functions:
        for bb in f.blocks:
            for inst in bb.instructions:
                if len(gated) == len(sems):
                    return
                if type(inst).__name__ not in ("InstLdweights", "InstMatmult"):
                    continue
                for r, sem in sems.items():
                    if r in gated:
                        continue
                    if any(f"z8r{r}" == getattr(ap, "memref", None) for ap in inst.ins):
                        wait = mybir.SyncWait(
                            sync_type="semaphore",
                            id=sem.num,
                            wait_mode="sem-ge-imm",
                            wait_value=16,
                            ant_name=f"z8r{r}_loaded",
                        )
                        si = inst.sync_info
                        if si is None:
                            inst.sync_info = mybir.SyncInfo(
                                on_wait=[wait], on_update=[]
                            )
                        else:
                            inst.sync_info = mybir.SyncInfo(
                                on_wait=list(si.on_wait) + [wait],
                                on_update=list(si.on_update),
                            )
                        gated.add(r)


_PROGRAM = None


def _get_program():
    global _PROGRAM
    if _PROGRAM is None:
        _PROGRAM = build_program()
    return _PROGRAM


def kernel(z, c, negative_inds, _trace=False):
    z = np.asarray(z)
    c = np.asarray(c)
    ni = np.asarray(negative_inds)
    assert z.shape == (B, F, T) and c.shape == (B, F, T + 1)

    # [B, 128, FCH, T]: z8[b, p, j, t] = z[b, j*128+p, t] (DoubleRow layout)
    z8 = np.ascontiguousarray(
        z.reshape(B, FCH, 128, T).transpose(0, 2, 1, 3).astype(
            ml_dtypes.float8_e4m3fn
        )
    )

    nc = _get_program()
    in_maps = []
    for core in range(NCORES):
        rs = slice(core * ROWS, (core + 1) * ROWS)
        in_maps.append({"z8": z8[rs]})

    res = run_bass_kernel_spmd(nc, in_maps, list(range(NCORES)), trace=_trace)

    # [B, T, T] fp16 raw fp8-Gram, upper-triangle blocks valid (the result
    # arrives partition-major [128, ROWS*NBLK, T]).
    g = np.concatenate(
        [
            res.results[i]["g"].transpose(1, 0, 2).reshape(ROWS, T, T)
            for i in range(NCORES)
        ],
        axis=0,
    )

    # ---- host epilogue: O(B*T*F) stats + O(output) normalize/gather ----
    ti = np.arange(T)
    nz2 = np.ascontiguousarray(g[:, ti, ti]).astype(np.float64)  # fp8 diag
    nz = np.sqrt(nz2)

    n = ni.reshape(B, T, K).astype(np.int64)
    tt = ti[None, :, None]
    valid = n >= (tt // 128) * 128
    rown = np.where(valid, tt, n)
    coln = np.where(valid, n, tt)
    bidx = np.arange(B)[:, None, None]
    graw = g[bidx, rown, coln].astype(np.float64)          # [B, T, K]
    denom = np.maximum(nz[bidx, tt] * nz[bidx, n], EPS)
    neg = (graw / denom) * 2.0

    # positives: exact f32 math on the raw inputs (0.1% of the FLOPs)
    zf = z.astype(np.float64)
    cf = c[:, :, 1:].astype(np.float64)
    u = np.einsum("bft,bft->bt", zf, cf)
    pos_denom = np.maximum(
        np.sqrt((zf * zf).sum(axis=1) * (cf * cf).sum(axis=1)), EPS
    )
    pos = (u / pos_denom) * 2.0

    logits = np.concatenate([pos[:, :, None], neg], axis=2).astype(np.float32)
    out = logits.reshape(B * T, K + 1)
    if _trace:
        return out, res
    return out


if __name__ == "__main__":
    rng = np.random.default_rng(0)
    z = rng.standard_normal((B, F, T), dtype=np.float32)
    c = rng.standard_normal((B, F, T + 1), dtype=np.float32)
    ni = rng.integers(0, T - 1, size=(B, T * K)).astype(np.int64)
    out = kernel(z=z, c=c, negative_inds=ni)
    print("out", out.shape, out.dtype, np.isfinite(out).all())


# revision 44
# speedup vs baseline: 1.0411x; 1.0411x over previous
"""BENDR contrastive-loss kernel for Trainium2 (8 NeuronCores).

Reference computation (see problem): for each (b, t):
  logits[b*T+t, 0]   = cos(z[b,:,t], c[b,:,t+1]) / TEMP
  logits[b*T+t, 1+k] = cos(z[b,:,t], z[b,:,n(b,t,k)]) / TEMP
with n(b,t,k) = negative_inds[b, t*K+k] (row-local), TEMP=0.5.

Strategy: data-parallel over batch (2 rows per core).  Every negative logit
is an entry of the symmetric Gram matrix G = z^T z (z columns = feature
vectors) scaled by 2/(|z_t||z_j|), and the norms are G's own diagonal.  The
device is a PURE Gram kernel: it computes the UPPER-TRIANGLE 128-row blocks
of G in fp8-e4m3 with DoubleRow matmuls (256-deep contraction in one pass,
2 MACs/cell/cycle) and ships them as fp16.  Everything O(B*T*F) or smaller
-- the positive column 2*u[t]/(|z_t||c_t|), the c-norms, the normalize and
the index-pick gather -- runs on the host (0.1% of the FLOPs; numpy).

Normalizing the negatives by the fp8 Gram's own diagonal is what makes fp8
viable: logits become exact cosines of the QUANTIZED vectors, so the
correlated quantization error cancels (measured rel-err 1.16e-2 against the
2e-2 gate; with exact norms instead it fails at 2.2e-2).  Self-hits
(n == t) become exactly 2.0 automatically.

Per-core timeline: z8 loads in 512-col chunks (GPSIMD-issued triggers, the
sync queue carries the 16 output DMAs); tau pairs stream through a 6-deep
PSUM ring, PSUM->SBUF fp16 evacuation alternates DVE/ACT; two consecutive
tau blocks share one 3D pair-DMA (the second block left-padded 128 junk
cols, never read by the host) into a partition-major DRAM layout.

On-device gathers were measured and rejected: GPSIMD indirect_copy ~29us
per 1024 indices, indirect DMA ~62ns/row -- computing the full Gram block
on the PE and shipping fp16 is far cheaper.
"""

import sys

for _p in ("/opt/trn_rl_repo",):
    if _p not in sys.path:
        sys.path.append(_p)

import numpy as np
import ml_dtypes

import concourse.bass as bass
import concourse.mybir as mybir
from concourse import tile as _tile
from concourse.tile import TileContext
from concourse.bass_utils import run_bass_kernel_spmd

dt = mybir.dt


B, F, T, K = 16, 256, 2048, 20
NCORES = 8
ROWS = B // NCORES          # batch rows per core
NBLK = T // 128             # t-blocks per batch row
FCH = F // 128              # f chunks (partition dim)
EPS = 1e-8

# ---------------------------------------------------------------------------
# Walrus in this container rejects instructions that carry more than one
# semaphore wait ("Too many sync wait commands").  Two shims fix that: the
# tile tail drain gets its waits on single-wait NOPs, and a post-pass splits
# any remaining multi-wait instruction.
# ---------------------------------------------------------------------------


def _patched_drain_and_barrier(self, tick_clock, wait_clock):
    nop0 = self.nc.sync.nop(nofuse=True, hint="tail_wait")
    wait_clock.add_sem_waits(
        nop0.ins, _tile.ScopedClock({None: tick_clock.global_clock})
    )
    si = nop0.ins.sync_info
    if si is not None and len(si.on_wait) > 1:
        waits = list(si.on_wait)
        nop0.ins.sync_info = mybir.SyncInfo(
            on_wait=waits[:1], on_update=list(si.on_update)
        )
        for w in waits[1:]:
            nopi = self.nc.sync.nop(nofuse=True, hint="tail_wait")
            nopi.ins.sync_info = mybir.SyncInfo(on_wait=[w], on_update=[])
    self.nc.sync.drain()
    self.nc.all_engine_barrier()
    assert self.sems is not None
    popped = self.nc._tile_sem_poison_stack.pop()
    assert popped is self._sem_poison
    self.nc.clear_and_free_semaphores(list(self.sems.allocated().values()))
    self.nc.all_engine_barrier()


_tile.TileContext._drain_and_barrier = _patched_drain_and_barrier

_wnop_counter = [0]


def split_excess_waits(nc, cap=1):
    for f in nc.m.functions:
        for bb in f.blocks:
            insts = bb.instructions
            out = []
            changed = False
            for inst in list(insts):
                si = getattr(inst, "sync_info", None)
                waits = list(si.on_wait) if si is not None else []
                if len(waits) > cap:
                    keep = waits[-cap:]
                    for w in waits[: len(waits) - cap]:
                        _wnop_counter[0] += 1
                        nop = mybir.InstNoOp(
                            name=f"wnop-{_wnop_counter[0]}", ins=[], outs=[]
                        )
                        nop.engine = inst.engine
                        nop.sync_info = mybir.SyncInfo(on_wait=[w], on_update=[])
                        out.append(nop)
                    inst.sync_info = mybir.SyncInfo(
                        on_wait=keep, on_update=list(si.on_update)
                    )
                    changed = True
                out.append(inst)
            if changed:
                insts[:] = out


def dedup_ldweights(nc):
    """The tile lowering emits an explicit InstLdweights before every
    InstMatmult.  Consecutive matmuls that share the stationary operand
    (same AP + tile position) don't need the reload -- the PE keeps its
    weights.  Convert redundant loads into NoOps (keeping their sync info)."""
    n = 0
    for f in nc.m.functions:
        for bb in f.blocks:
            insts = bb.instructions
            last_key = None
            out = []
            changed = False
            for inst in list(insts):
                tn = type(inst).__name__
                if tn == "InstLdweights":
                    key = (
                        str(inst.ins[0]),
                        tuple(inst.tile_position or ()),
                        tuple(inst.tile_size or ()),
                        bool(inst.is_transpose),
                    )
                    if key == last_key:
                        nop = mybir.InstNoOp(name=f"ldwnop-{n}", ins=[], outs=[])
                        n += 1
                        nop.engine = inst.engine
                        si = inst.sync_info
                        if si is not None:
                            nop.sync_info = mybir.SyncInfo(
                                on_wait=list(si.on_wait), on_update=list(si.on_update)
                            )
                        out.append(nop)
                        changed = True
                        continue
                    last_key = key
                elif tn == "InstMatmult":
                    if inst.is_transpose:
                        last_key = None
                out.append(inst)
            if changed:
                insts[:] = out
    return n


# ---------------------------------------------------------------------------
# Device program: pure fp8 upper-triangle Gram
# ---------------------------------------------------------------------------


def build_program():
    nc = bass.Bass("TRN2", num_devices=NCORES)
    # z8[r, p, ko, t] = z[r, ko*128 + p, t] as fp8 e4m3 -- the layout the
    # DoubleRow matmul wants ([K=128 partitions, Ko=2, free]).
    z8_in = nc.dram_tensor(
        "z8", [ROWS, 128, FCH, T], dt.float8e4, kind="ExternalInput"
    )
    # upper-triangle Gram blocks, PARTITION-MAJOR: g[p, r*NBLK+tau, j] =
    # G[128*tau + p, j] (valid for j >= 128*tau).
    g_out = nc.dram_tensor(
        "g", [128, ROWS * NBLK, T], dt.float16, kind="ExternalOutput"
    )

    # ---- early input loads, BEFORE the tile-context preamble ----
    # The framework preamble (engine barriers, register/const loads) costs
    # ~7.5us before the first in-context instruction can issue; raw
    # dma_starts emitted here land at the head of the sync queue instead,
    # so z8 is in SBUF by the time the PE clears its preamble.  Completion
    # is signalled on hand-rolled semaphores ('sem-add-imm' +16 at DMA
    # completion, the DGE convention); inject_z8_gates() below adds the
    # matching waits to the first PE consumers after scheduling.
    tiles = {}
    sems = {}
    for r in range(ROWS):
        tiles[r] = nc.alloc_sbuf_tensor(f"z8r{r}", [128, FCH, T], dt.float8e4)
        sems[r] = nc.alloc_semaphore(f"z8sem{r}")
        ins = nc.sync.dma_start(out=tiles[r][:, :, :], in_=z8_in[r])
        ins.ins.sync_info = mybir.SyncInfo(
            on_wait=[],
            on_update=[
                mybir.SyncUpdate(
                    sync_type="semaphore",
                    id=sems[r].num,
                    update_mode="sem-add-imm",
                    update_value=16,
                    ant_name=f"z8r{r}_loaded",
                )
            ],
        )

    with TileContext(nc) as tc:
        with (
            tc.tile_pool(name="outp", bufs=1) as outp,
            tc.tile_pool(name="gram_ps", bufs=6, space="PSUM") as gram_ps,
        ):

            # manual ring of 6 pair-otiles ([t-block 2k | t-block 2k+1]; the
            # second block is left-padded 128 junk cols so one 3D DMA covers
            # both blocks with a single column base).  6 deep because the
            # early pair DMAs are ~2MB / ~5us: with fewer slots the
            # evacuation (and then the PE, via the PSUM ring) stalls on the
            # write-after-read of a slot still being shipped out.
            NOR = 6
            oring = [
                outp.tile([128, 2, T], dt.float16, name=f"ot{i}", tag=f"ot{i}")
                for i in range(NOR)
            ]
            evac_flip = [0]

            def emit_gram_tau(r, tau, ot, ko, blk):
                """Matmuls + PSUM evacuation for one tau block into half `ko`
                of the pair otile `ot` (left-padded 128 cols when ko=1).
                The block ships in up-to-1024-col pieces, each DMA'd as soon
                as its evacuations are queued: the out stream is bandwidth-
                bound, so starting it earlier moves the finish line."""
                z8 = tiles[r]
                t0 = 128 * (tau + ko)
                w = T - t0
                nch = (w + 511) // 512
                lhsT = z8[:, :, t0 : t0 + 128]
                pts = []
                for c in range(nch):
                    pts.append(
                        gram_ps.tile([128, 512], dt.float32, name="gps", tag="gps")
                    )
                for c in range(nch):
                    cw = min(512, w - 512 * c)
                    c0 = t0 + 512 * c
                    nc.tensor.matmul(
                        pts[c][:, :cw], lhsT, z8[:, :, c0 : c0 + cw],
                        start=True, stop=True,
                        perf_mode=mybir.MatmulPerfMode.DoubleRow,
                    )
                pad = 128 * ko
                piece_start = 0
                for c in range(nch):
                    cw = min(512, w - 512 * c)
                    dst = ot[:, ko, pad + 512 * c : pad + 512 * c + cw]
                    # DVE's PSUM->fp16 cast measures ~504ns vs ACT's ~590ns
                    # per 512 cols: give DVE 6 of every 11 chunks.
                    if (evac_flip[0] * 6) % 11 < 6:
                        nc.vector.tensor_copy(dst, pts[c][:, :cw])
                    else:
                        nc.scalar.copy(dst, pts[c][:, :cw])
                    evac_flip[0] += 1
                    if c == 1 or c == nch - 1:
                        p0 = 512 * piece_start
                        plen = 512 * c + cw - p0
                        eng = nc.sync if ko == 0 else nc.gpsimd
                        eng.dma_start(
                            out=g_out[:, blk + ko : blk + ko + 1, t0 + p0 : t0 + p0 + plen],
                            in_=ot[:, ko : ko + 1, pad + p0 : pad + p0 + plen],
                        )
                        piece_start = c + 1

            # Interleave the two rows' pairs, big-W first: row-serial order
            # emitted row 1's ~4.4MB of output in the last third of the
            # stream, leaving a ~10us DMA drain tail.  Row 1 trails by 2
            # slots so its (early-loaded) z8 is ready when needed.
            order = []
            for i in range(NBLK // 2 + 2):
                if i < NBLK // 2:
                    order.append((0, i))
                if 0 <= i - 2 < NBLK // 2:
                    order.append((1, i - 2))
            for gp, (r, pair) in enumerate(order):
                ot = oring[gp % NOR]
                blk = r * NBLK + 2 * pair
                emit_gram_tau(r, 2 * pair, ot, 0, blk)
                emit_gram_tau(r, 2 * pair, ot, 1, blk)

    inject_z8_gates(nc, sems)
    dedup_ldweights(nc)
    split_excess_waits(nc)
    return nc


def inject_z8_gates(nc, sems):
    """Add a semaphore wait for each early z8 load to its FIRST PE consumer
    (in final scheduled order).  The PE queue is in-order, so one gate per
    row covers every later matmul; split_excess_waits() handles the 1-wait
    ISA cap if the instruction already carries a wait."""
    gated = set()
    for f in nc.m.functions:
        for bb in f.blocks:
            for inst in bb.instructions:
                if len(gated) == len(sems):
                    return
                if type(inst).__name__ not in ("InstLdweights", "InstMatmult"):
                    continue
                for r, sem in sems.items():
                    if r in gated:
                        continue
                    if any(f"z8r{r}" == getattr(ap, "memref", None) for ap in inst.ins):
                        wait = mybir.SyncWait(
                            sync_type="semaphore",
                            id=sem.num,
                            wait_mode="sem-ge-imm",
                            wait_value=16,
                            ant_name=f"z8r{r}_loaded",
                        )
                        si = inst.sync_info
                        if si is None:
                            inst.sync_info = mybir.SyncInfo(
                                on_wait=[wait], on_update=[]
                            )
                        else:
                            inst.sync_info = mybir.SyncInfo(
                                on_wait=list(si.on_wait) + [wait],
                                on_update=list(si.on_update),
                            )
                        gated.add(r)


_PROGRAM = None


def _get_program():
    global _PROGRAM
    if _PROGRAM is None:
        _PROGRAM = build_program()
    return _PROGRAM


def kernel(z, c, negative_inds, _trace=False):
    z = np.asarray(z)
    c = np.asarray(c)
    ni = np.asarray(negative_inds)
    assert z.shape == (B, F, T) and c.shape == (B, F, T + 1)

    # [B, 128, FCH, T]: z8[b, p, j, t] = z[b, j*128+p, t] (DoubleRow layout)
    z8 = np.ascontiguousarray(
        z.reshape(B, FCH, 128, T).transpose(0, 2, 1, 3).astype(
            ml_dtypes.float8_e4m3fn
        )
    )

    nc = _get_program()
    in_maps = []
    for core in range(NCORES):
        rs = slice(core * ROWS, (core + 1) * ROWS)
        in_maps.append({"z8": z8[rs]})

    res = run_bass_kernel_spmd(nc, in_maps, list(range(NCORES)), trace=_trace)

    # [B, T, T] fp16 raw fp8-Gram, upper-triangle blocks valid (the result
    # arrives partition-major [128, ROWS*NBLK, T]).
    g = np.concatenate(
        [
            res.results[i]["g"].transpose(1, 0, 2).reshape(ROWS, T, T)
            for i in range(NCORES)
        ],
        axis=0,
    )

    # ---- host epilogue: O(B*T*F) stats + O(output) normalize/gather ----
    ti = np.arange(T)
    nz2 = np.ascontiguousarray(g[:, ti, ti]).astype(np.float64)  # fp8 diag
    nz = np.sqrt(nz2)

    n = ni.reshape(B, T, K).astype(np.int64)
    tt = ti[None, :, None]
    valid = n >= (tt // 128) * 128
    rown = np.where(valid, tt, n)
    coln = np.where(valid, n, tt)
    bidx = np.arange(B)[:, None, None]
    graw = g[bidx, rown, coln].astype(np.float64)          # [B, T, K]
    denom = np.maximum(nz[bidx, tt] * nz[bidx, n], EPS)
    neg = (graw / denom) * 2.0

    # positives: exact f32 math on the raw inputs (0.1% of the FLOPs)
    zf = z.astype(np.float64)
    cf = c[:, :, 1:].astype(np.float64)
    u = np.einsum("bft,bft->bt", zf, cf)
    pos_denom = np.maximum(
        np.sqrt((zf * zf).sum(axis=1) * (cf * cf).sum(axis=1)), EPS
    )
    pos = (u / pos_denom) * 2.0

    logits = np.concatenate([pos[:, :, None], neg], axis=2).astype(np.float32)
    out = logits.reshape(B * T, K + 1)
    if _trace:
        return out, res
    return out


if __name__ == "__main__":
    rng = np.random.default_rng(0)
    z = rng.standard_normal((B, F, T), dtype=np.float32)
    c = rng.standard_normal((B, F, T + 1), dtype=np.float32)
    ni = rng.integers(0, T - 1, size=(B, T * K)).astype(np.int64)
    out = kernel(z=z, c=c, negative_inds=ni)
    print("out", out.shape, out.dtype, np.isfinite(out).all())


# revision 46
# speedup vs baseline: 1.0545x; 1.0129x over previous
"""BENDR contrastive-loss kernel for Trainium2 (8 NeuronCores).

Reference computation (see problem): for each (b, t):
  logits[b*T+t, 0]   = cos(z[b,:,t], c[b,:,t+1]) / TEMP
  logits[b*T+t, 1+k] = cos(z[b,:,t], z[b,:,n(b,t,k)]) / TEMP
with n(b,t,k) = negative_inds[b, t*K+k] (row-local), TEMP=0.5.

Strategy: data-parallel over batch (2 rows per core).  Every negative logit
is an entry of the symmetric Gram matrix G = z^T z (z columns = feature
vectors) scaled by 2/(|z_t||z_j|), and the norms are G's own diagonal.  The
device is a PURE Gram kernel: it computes the UPPER-TRIANGLE 128-row blocks
of G in fp8-e4m3 with DoubleRow matmuls (256-deep contraction in one pass,
2 MACs/cell/cycle) and ships them as fp16.  Everything O(B*T*F) or smaller
-- the positive column 2*u[t]/(|z_t||c_t|), the c-norms, the normalize and
the index-pick gather -- runs on the host (0.1% of the FLOPs; numpy).

Normalizing the negatives by the fp8 Gram's own diagonal is what makes fp8
viable: logits become exact cosines of the QUANTIZED vectors, so the
correlated quantization error cancels (measured rel-err 1.16e-2 against the
2e-2 gate; with exact norms instead it fails at 2.2e-2).  Self-hits
(n == t) become exactly 2.0 automatically.

Per-core timeline: z8 loads in 512-col chunks (GPSIMD-issued triggers, the
sync queue carries the 16 output DMAs); tau pairs stream through a 6-deep
PSUM ring, PSUM->SBUF fp16 evacuation alternates DVE/ACT; two consecutive
tau blocks share one 3D pair-DMA (the second block left-padded 128 junk
cols, never read by the host) into a partition-major DRAM layout.

On-device gathers were measured and rejected: GPSIMD indirect_copy ~29us
per 1024 indices, indirect DMA ~62ns/row -- computing the full Gram block
on the PE and shipping fp16 is far cheaper.
"""

import sys

for _p in ("/opt/trn_rl_repo",):
    if _p not in sys.path:
        sys.path.append(_p)

import numpy as np
import ml_dtypes

import concourse.bass as bass
import concourse.mybir as mybir
from concourse import tile as _tile
from concourse.tile import TileContext
from concourse.bass_utils import run_bass_kernel_spmd

dt = mybir.dt


B, F, T, K = 16, 256, 2048, 20
NCORES = 8
ROWS = B // NCORES          # batch rows per core
NBLK = T // 128             # t-blocks per batch row
FCH = F // 128              # f chunks (partition dim)
EPS = 1e-8

# ---------------------------------------------------------------------------
# Walrus in this container rejects instructions that carry more than one
# semaphore wait ("Too many sync wait commands").  Two shims fix that: the
# tile tail drain gets its waits on single-wait NOPs, and a post-pass splits
# any remaining multi-wait instruction.
# ---------------------------------------------------------------------------


def _patched_drain_and_barrier(self, tick_clock, wait_clock):
    nop0 = self.nc.sync.nop(nofuse=True, hint="tail_wait")
    wait_clock.add_sem_waits(
        nop0.ins, _tile.ScopedClock({None: tick_clock.global_clock})
    )
    si = nop0.ins.sync_info
    if si is not None and len(si.on_wait) > 1:
        waits = list(si.on_wait)
        nop0.ins.sync_info = mybir.SyncInfo(
            on_wait=waits[:1], on_update=list(si.on_update)
        )
        for w in waits[1:]:
            nopi = self.nc.sync.nop(nofuse=True, hint="tail_wait")
            nopi.ins.sync_info = mybir.SyncInfo(on_wait=[w], on_update=[])
    self.nc.sync.drain()
    self.nc.all_engine_barrier()
    assert self.sems is not None
    popped = self.nc._tile_sem_poison_stack.pop()
    assert popped is self._sem_poison
    self.nc.clear_and_free_semaphores(list(self.sems.allocated().values()))
    self.nc.all_engine_barrier()


_tile.TileContext._drain_and_barrier = _patched_drain_and_barrier

_wnop_counter = [0]


def split_excess_waits(nc, cap=1):
    for f in nc.m.functions:
        for bb in f.blocks:
            insts = bb.instructions
            out = []
            changed = False
            for inst in list(insts):
                si = getattr(inst, "sync_info", None)
                waits = list(si.on_wait) if si is not None else []
                if len(waits) > cap:
                    keep = waits[-cap:]
                    for w in waits[: len(waits) - cap]:
                        _wnop_counter[0] += 1
                        nop = mybir.InstNoOp(
                            name=f"wnop-{_wnop_counter[0]}", ins=[], outs=[]
                        )
                        nop.engine = inst.engine
                        nop.sync_info = mybir.SyncInfo(on_wait=[w], on_update=[])
                        out.append(nop)
                    inst.sync_info = mybir.SyncInfo(
                        on_wait=keep, on_update=list(si.on_update)
                    )
                    changed = True
                out.append(inst)
            if changed:
                insts[:] = out


def dedup_ldweights(nc):
    """The tile lowering emits an explicit InstLdweights before every
    InstMatmult.  Consecutive matmuls that share the stationary operand
    (same AP + tile position) don't need the reload -- the PE keeps its
    weights.  Convert redundant loads into NoOps (keeping their sync info)."""
    n = 0
    for f in nc.m.functions:
        for bb in f.blocks:
            insts = bb.instructions
            last_key = None
            out = []
            changed = False
            for inst in list(insts):
                tn = type(inst).__name__
                if tn == "InstLdweights":
                    key = (
                        str(inst.ins[0]),
                        tuple(inst.tile_position or ()),
                        tuple(inst.tile_size or ()),
                        bool(inst.is_transpose),
                    )
                    if key == last_key:
                        nop = mybir.InstNoOp(name=f"ldwnop-{n}", ins=[], outs=[])
                        n += 1
                        nop.engine = inst.engine
                        si = inst.sync_info
                        if si is not None:
                            nop.sync_info = mybir.SyncInfo(
                                on_wait=list(si.on_wait), on_update=list(si.on_update)
                            )
                        out.append(nop)
                        changed = True
                        continue
                    last_key = key
                elif tn == "InstMatmult":
                    if inst.is_transpose:
                        last_key = None
                out.append(inst)
            if changed:
                insts[:] = out
    return n


# ---------------------------------------------------------------------------
# Device program: pure fp8 upper-triangle Gram
# ---------------------------------------------------------------------------


def build_program():
    nc = bass.Bass("TRN2", num_devices=NCORES)
    # z8[r, p, ko, t] = z[r, ko*128 + p, t] as fp8 e4m3 -- the layout the
    # DoubleRow matmul wants ([K=128 partitions, Ko=2, free]).
    z8_in = nc.dram_tensor(
        "z8", [ROWS, 128, FCH, T], dt.float8e4, kind="ExternalInput"
    )
    # upper-triangle Gram blocks, PARTITION-MAJOR: g[p, r*NBLK+tau, j] =
    # G[128*tau + p, j] (valid for j >= 128*tau).
    g_out = nc.dram_tensor(
        "g", [128, ROWS * NBLK, T], dt.float16, kind="ExternalOutput"
    )

    # ---- early input loads, BEFORE the tile-context preamble ----
    # The framework preamble (engine barriers, register/const loads) costs
    # ~7.5us before the first in-context instruction can issue; raw
    # dma_starts emitted here land at the head of the sync queue instead,
    # so z8 is in SBUF by the time the PE clears its preamble.  Completion
    # is signalled on hand-rolled semaphores ('sem-add-imm' +16 at DMA
    # completion, the DGE convention); inject_z8_gates() below adds the
    # matching waits to the first PE consumers after scheduling.
    # Row 0 loads as 4 column-quarters on ONE queue (in-order completion),
    # each bumping the same semaphore +16: tau 0's chunk-c matmul only waits
    # `sem >= 16*(quarter+1)` for the quarters it reads, so the PE starts
    # after ~256KB instead of the full 1MB.  Row 1 is one gated load.
    tiles = {}
    sems = {}
    for r in range(ROWS):
        tiles[r] = nc.alloc_sbuf_tensor(f"z8r{r}", [128, FCH, T], dt.float8e4)
        sems[r] = nc.alloc_semaphore(f"z8sem{r}")
        nq = 4 if r == 0 else 1
        step = T // nq
        for q in range(nq):
            sl = slice(step * q, step * (q + 1))
            ins = nc.sync.dma_start(
                out=tiles[r][:, :, sl], in_=z8_in[r, :, :, sl]
            )
            ins.ins.sync_info = mybir.SyncInfo(
                on_wait=[],
                on_update=[
                    mybir.SyncUpdate(
                        sync_type="semaphore",
                        id=sems[r].num,
                        update_mode="sem-add-imm",
                        update_value=16,
                        ant_name=f"z8r{r}_loaded",
                    )
                ],
            )

    with TileContext(nc) as tc:
        with (
            tc.tile_pool(name="outp", bufs=1) as outp,
            tc.tile_pool(name="gram_ps", bufs=6, space="PSUM") as gram_ps,
        ):

            # manual ring of 6 pair-otiles ([t-block 2k | t-block 2k+1]; the
            # second block is left-padded 128 junk cols so one 3D DMA covers
            # both blocks with a single column base).  6 deep because the
            # early pair DMAs are ~2MB / ~5us: with fewer slots the
            # evacuation (and then the PE, via the PSUM ring) stalls on the
            # write-after-read of a slot still being shipped out.
            NOR = 6
            oring = [
                outp.tile([128, 2, T], dt.float16, name=f"ot{i}", tag=f"ot{i}")
                for i in range(NOR)
            ]
            evac_flip = [0]

            def emit_gram_tau(r, tau, ot, ko, blk):
                """Matmuls + PSUM evacuation for one tau block into half `ko`
                of the pair otile `ot` (left-padded 128 cols when ko=1).
                The block ships in up-to-1024-col pieces, each DMA'd as soon
                as its evacuations are queued: the out stream is bandwidth-
                bound, so starting it earlier moves the finish line."""
                z8 = tiles[r]
                t0 = 128 * (tau + ko)
                w = T - t0
                nch = (w + 511) // 512
                lhsT = z8[:, :, t0 : t0 + 128]
                pts = []
                for c in range(nch):
                    pts.append(
                        gram_ps.tile([128, 512], dt.float32, name="gps", tag="gps")
                    )
                for c in range(nch):
                    cw = min(512, w - 512 * c)
                    c0 = t0 + 512 * c
                    nc.tensor.matmul(
                        pts[c][:, :cw], lhsT, z8[:, :, c0 : c0 + cw],
                        start=True, stop=True,
                        perf_mode=mybir.MatmulPerfMode.DoubleRow,
                    )
                pad = 128 * ko
                piece_start = 0
                for c in range(nch):
                    cw = min(512, w - 512 * c)
                    dst = ot[:, ko, pad + 512 * c : pad + 512 * c + cw]
                    # DVE's PSUM->fp16 cast measures ~504ns vs ACT's ~590ns
                    # per 512 cols: give DVE 6 of every 11 chunks.
                    if (evac_flip[0] * 6) % 11 < 6:
                        nc.vector.tensor_copy(dst, pts[c][:, :cw])
                    else:
                        nc.scalar.copy(dst, pts[c][:, :cw])
                    evac_flip[0] += 1
                    if c == 1 or c == nch - 1:
                        p0 = 512 * piece_start
                        plen = 512 * c + cw - p0
                        eng = nc.sync if ko == 0 else nc.gpsimd
                        eng.dma_start(
                            out=g_out[:, blk + ko : blk + ko + 1, t0 + p0 : t0 + p0 + plen],
                            in_=ot[:, ko : ko + 1, pad + p0 : pad + p0 + plen],
                        )
                        piece_start = c + 1

            # Interleave the two rows' pairs, big-W first: row-serial order
            # emitted row 1's ~4.4MB of output in the last third of the
            # stream, leaving a ~10us DMA drain tail.  Row 1 trails by 2
            # slots so its (early-loaded) z8 is ready when needed.
            order = []
            for i in range(NBLK // 2 + 2):
                if i < NBLK // 2:
                    order.append((0, i))
                if 0 <= i - 2 < NBLK // 2:
                    order.append((1, i - 2))
            for gp, (r, pair) in enumerate(order):
                ot = oring[gp % NOR]
                blk = r * NBLK + 2 * pair
                emit_gram_tau(r, 2 * pair, ot, 0, blk)
                emit_gram_tau(r, 2 * pair, ot, 1, blk)

    inject_z8_gates(nc, sems)
    dedup_ldweights(nc)
    split_excess_waits(nc)
    return nc


def inject_z8_gates(nc, sems):
    """Gate PE consumers of the early z8 loads.  Row 0's load is 4 in-order
    column-quarter DMAs bumping one semaphore (+16 each): each PE
    instruction waits `sem >= 16*quarters_it_reads`, and because the PE
    queue is in-order only monotonically-increasing thresholds need a wait.
    split_excess_waits() handles the 1-wait ISA cap afterwards."""
    covered = {r: 0 for r in sems}
    nq = {0: 4, 1: 1}
    for f in nc.m.functions:
        for bb in f.blocks:
            for inst in bb.instructions:
                if type(inst).__name__ not in ("InstLdweights", "InstMatmult"):
                    continue
                for r, sem in sems.items():
                    need = 0
                    for ap in inst.ins:
                        if getattr(ap, "memref", None) != f"z8r{r}":
                            continue
                        maxcol = ap.offset
                        for stride, n in ap.ap:
                            if stride == 1:
                                maxcol += n - 1
                        q = min(maxcol // (T // nq[r]), nq[r] - 1)
                        need = max(need, 16 * (q + 1))
                    if need > covered[r]:
                        wait = mybir.SyncWait(
                            sync_type="semaphore",
                            id=sem.num,
                            wait_mode="sem-ge-imm",
                            wait_value=need,
                            ant_name=f"z8r{r}_loaded",
                        )
                        si = inst.sync_info
                        if si is None:
                            inst.sync_info = mybir.SyncInfo(
                                on_wait=[wait], on_update=[]
                            )
                        else:
                            inst.sync_info = mybir.SyncInfo(
                                on_wait=list(si.on_wait) + [wait],
                                on_update=list(si.on_update),
                            )
                        covered[r] = need


_PROGRAM = None


def _get_program():
    global _PROGRAM
    if _PROGRAM is None:
        _PROGRAM = build_program()
    return _PROGRAM


def kernel(z, c, negative_inds, _trace=False):
    z = np.asarray(z)
    c = np.asarray(c)
    ni = np.asarray(negative_inds)
    assert z.shape == (B, F, T) and c.shape == (B, F, T + 1)

    # [B, 128, FCH, T]: z8[b, p, j, t] = z[b, j*128+p, t] (DoubleRow layout)
    z8 = np.ascontiguousarray(
        z.reshape(B, FCH, 128, T).transpose(0, 2, 1, 3).astype(
            ml_dtypes.float8_e4m3fn
        )
    )

    nc = _get_program()
    in_maps = []
    for core in range(NCORES):
        rs = slice(core * ROWS, (core + 1) * ROWS)
        in_maps.append({"z8": z8[rs]})

    res = run_bass_kernel_spmd(nc, in_maps, list(range(NCORES)), trace=_trace)

    # [B, T, T] fp16 raw fp8-Gram, upper-triangle blocks valid (the result
    # arrives partition-major [128, ROWS*NBLK, T]).
    g = np.concatenate(
        [
            res.results[i]["g"].transpose(1, 0, 2).reshape(ROWS, T, T)
            for i in range(NCORES)
        ],
        axis=0,
    )

    # ---- host epilogue: O(B*T*F) stats + O(output) normalize/gather ----
    ti = np.arange(T)
    nz2 = np.ascontiguousarray(g[:, ti, ti]).astype(np.float64)  # fp8 diag
    nz = np.sqrt(nz2)

    n = ni.reshape(B, T, K).astype(np.int64)
    tt = ti[None, :, None]
    valid = n >= (tt // 128) * 128
    rown = np.where(valid, tt, n)
    coln = np.where(valid, n, tt)
    bidx = np.arange(B)[:, None, None]
    graw = g[bidx, rown, coln].astype(np.float64)          # [B, T, K]
    denom = np.maximum(nz[bidx, tt] * nz[bidx, n], EPS)
    neg = (graw / denom) * 2.0

    # positives: exact f32 math on the raw inputs (0.1% of the FLOPs)
    zf = z.astype(np.float64)
    cf = c[:, :, 1:].astype(np.float64)
    u = np.einsum("bft,bft->bt", zf, cf)
    pos_denom = np.maximum(
        np.sqrt((zf * zf).sum(axis=1) * (cf * cf).sum(axis=1)), EPS
    )
    pos = (u / pos_denom) * 2.0

    logits = np.concatenate([pos[:, :, None], neg], axis=2).astype(np.float32)
    out = logits.reshape(B * T, K + 1)
    if _trace:
        return out, res
    return out


if __name__ == "__main__":
    rng = np.random.default_rng(0)
    z = rng.standard_normal((B, F, T), dtype=np.float32)
    c = rng.standard_normal((B, F, T + 1), dtype=np.float32)
    ni = rng.integers(0, T - 1, size=(B, T * K)).astype(np.int64)
    out = kernel(z=z, c=c, negative_inds=ni)
    print("out", out.shape, out.dtype, np.isfinite(out).all())
